# revision 43
# baseline (speedup 1.0000x reference)
"""MixProp GNN message-passing kernel for 8 TRN2 NeuronCores.

Reference computation (per batch element b):
    A_n = row_normalize(A + I)
    H_0 = X;  H_k = beta*X + (1-beta) * A_n @_nodes H_{k-1}   (k=1..3)
    out = W @_channels concat([H_0..H_3]) + bias

Kernel strategy:
  - Data-parallel over batch: B=8 batch elements -> 8 cores, no collectives.
  - Host precomputes G_k = polynomial in A_n s.t. H_k = G_k @ X (node-mixing
    and channel-mixing commute, and the hop recurrence is affine in X).
  - Host pre-transposes + pre-casts X to [w, l, c] bf16 so the device DMA
    is fully contiguous and lhsT slices [128, 32] are contiguous.
  - On device, for each seq position l (all ops are pointwise in l):
      * 4 column-packed matmuls (tile_position=(0,32j)) per 128-node block
        of the contraction build a PSUM tile H0[(src,ch), v] where src =
        (X, G1X, G2X, G3X): lhsT = X[:, l-slice] (stationary), rhs =
        I / G_k^T (moving). The four column groups run concurrently on the
        PE's 32-column strips, so the X "transpose" (identity matmul,
        group 0) adds no wall time.
      * conv matmuls vs W^T produce out[(vh,o), v-half] in psum partitions
        0:64 / 64:128; convs are batched (CB seq positions per batch,
        2 l's per PSUM bank) so the W stationary swap cost (~110 ns
        drain+LDW) is paid once per batch, not once per l.
  - PSUM->SBUF evacuation of H0 on DVE; conv bias-add on ScalarE.
  - Output DRAM layout is [vh, o, l, v] so both the ScalarE bias-write and
    the store DMA are fully contiguous (strided 128B-run stores measured
    only ~114 GB/s); the host transposes back to [o, v, l] for free.
"""

import sys

sys.path.insert(0, "/opt/trn_rl_repo")

import numpy as np

import concourse.bass as bass
import concourse.bacc as bacc
import concourse.mybir as mybir
from concourse import tile
from concourse.tile import add_dep_helper
from concourse import bass_utils

GDEP = 3
BETA = 0.05
C_IN = 32
C_OUT = 64
N = 512
B = 8
L = 256
NB = N // 128  # node blocks of 128
NSRC = GDEP + 1  # (X, G1X, G2X, G3X)

F32 = mybir.dt.float32


class CFG:
    def __init__(self, L=L, Lc=32, mm_dt=mybir.dt.bfloat16):
        assert L % Lc == 0
        self.L = L
        self.Lc = Lc
        self.mm_dt = mm_dt


def body(nc, tc, outs, ins, cfg: CFG):
    """Emit the per-core program. ins/outs are dicts of DRAM APs."""
    X2_d = ins["x2"]        # [N, L, C_IN] bf16  pre-transposed X
    GI_d = ins["gi"]        # [NB, 128, GDEP, N] bf16: per wb (G1^T..G3^T)
    I_d = ins["ident"]      # [128, 128] bf16 identity
    W_d = ins["wt"]         # [128, C_OUT] bf16  W^T
    b_d = ins["bias2"]      # [128, 1]     f32   bias duplicated for (vh, o)
    out_d = outs["out"]     # [2, C_OUT, L, 256] f32: (vh, o, l, v-half)

    Lc = cfg.Lc
    mm_dt = cfg.mm_dt
    n_chunks = cfg.L // Lc

    CB = 12  # conv batch: l's whose convs share one W LDWEIGHTS pair

    with (
        tc.tile_pool(name="const", bufs=1) as cpool,
        tc.tile_pool(name="xsb", bufs=3) as xsb_pool,
        tc.tile_pool(name="h0sb", bufs=2 * CB) as h0sb_pool,
        tc.tile_pool(name="outsb", bufs=6) as out_pool,
        tc.tile_pool(name="h0ps", bufs=3, space="PSUM") as h0ps_pool,
        tc.tile_pool(name="cvps", bufs=5, space="PSUM") as cvps_pool,
    ):
        # ---- first-chunk X loads + constants interleaved in urgency
        # order on the sync queue (the first matmul needs x(ch0,wb0) and
        # gi0 first; the priority-heap scheduler reorders same-priority
        # DMAs on other queues) ----
        gi_tiles = []
        w_t = cpool.tile([128, C_OUT], mm_dt, name="w_t")
        b_t = cpool.tile([128, 1], F32, name="b_t")

        def load_x(ch, eng, after=None):
            # X load: contiguous [128, Lc*C_IN] bf16 lines. Steady-state
            # loads issue from the otherwise-idle GpSimd engine (SWDGE)
            # one chunk ahead — a dma_start that waits on a slot
            # semaphore must not sit in an engine stream with critical
            # work behind it. `after` pins the load behind mid-chunk
            # compute so the scheduler can't hoist it into the
            # HBM-bandwidth-critical preamble.
            tiles = []
            for wb in range(NB):
                xsb = xsb_pool.tile(
                    [128, Lc * C_IN], mm_dt, name="xsb", tag=f"xsb{wb}"
                )
                dma = eng.dma_start(
                    xsb.rearrange("w (l c) -> w l c", c=C_IN),
                    X2_d[wb * 128:(wb + 1) * 128,
                         ch * Lc:(ch + 1) * Lc, :],
                )
                if after is not None:
                    add_dep_helper(
                        dma.ins, after.ins,
                        reason="pace X prefetch behind mid-chunk compute",
                    )
                tiles.append(xsb)
            return tiles

        # small-but-critical constants first: i128 gates the pc-first
        # identity matmul of every group
        i128 = cpool.tile([128, 128], mm_dt, name="i128")
        nc.sync.dma_start(i128[:], I_d[:])
        nc.sync.dma_start(w_t[:], W_d[:])
        nc.sync.dma_start(b_t[:], b_d[:])
        xsb_first = []
        for wb in range(NB):
            xsb = xsb_pool.tile(
                [128, Lc * C_IN], mm_dt, name="xsb", tag=f"xsb{wb}"
            )
            nc.sync.dma_start(
                xsb.rearrange("w (l c) -> w l c", c=C_IN),
                X2_d[wb * 128:(wb + 1) * 128, 0:Lc, :],
            )
            xsb_first.append(xsb)
            t = cpool.tile([128, GDEP * N], mm_dt, name=f"gi{wb}")
            nc.sync.dma_start(
                t.rearrange("p (s n) -> p s n", n=N), GI_d[wb]
            )
            gi_tiles.append(t)

        def gi_sl(wb, k):
            return gi_tiles[wb][:, k * N:(k + 1) * N]

        dst = out_d.rearrange("vh o l v -> (vh o) l v")

        xsb_next = xsb_first
        for ch in range(n_chunks):
            xsb_tiles = xsb_next

            # ---- per-seq-position pipeline, conv deferred one batch ----
            prev_batch = None
            batch = []
            for l0 in range(Lc):
                h0p = h0ps_pool.tile([128, N], F32, name="h0p")
                for wb in range(NB):
                    st = wb == 0
                    sp = wb == NB - 1
                    xl = xsb_tiles[wb][:, l0 * C_IN:(l0 + 1) * C_IN]
                    # X-transpose: single-shot N=128 identity matmul in
                    # column group 0 (writes only this wb's v-block; the
                    # shared I128 tile keeps the preamble DMA small).
                    nc.tensor.matmul(
                        h0p[0:32, wb * 128:(wb + 1) * 128], lhsT=xl,
                        rhs=i128[:], start=True, stop=True,
                        tile_position=(0, 0), skip_group_check=True,
                    )
                    for k in range(GDEP):
                        j = k + 1
                        nc.tensor.matmul(
                            h0p[32 * j:32 * (j + 1), :], lhsT=xl,
                            rhs=gi_sl(wb, k),
                            start=st, stop=sp, tile_position=(0, 32 * j),
                            skip_group_check=True,
                        )
                h0s = h0sb_pool.tile([128, N], mm_dt, name="h0s")
                cast_i = nc.vector.tensor_copy(out=h0s[:], in_=h0p[:])

                # prefetch the next chunk's X mid-chunk: early enough to
                # land in time, late enough not to steal HBM bandwidth
                # from the preamble-critical loads
                if l0 == Lc // 2 and ch + 1 < n_chunks:
                    xsb_next = load_x(ch + 1, nc.gpsimd, after=cast_i)

                batch.append((h0s, l0))
                if len(batch) == CB:
                    if prev_batch is not None:
                        _emit_conv_batch(
                            nc, cvps_pool, out_pool, w_t, b_t, dst,
                            ch * Lc, prev_batch,
                        )
                    prev_batch, batch = batch, []
            last = ch == n_chunks - 1
            _emit_conv_batch(
                nc, cvps_pool, out_pool, w_t, b_t, dst, ch * Lc,
                prev_batch, final=last and not batch,
            )
            if batch:
                _emit_conv_batch(
                    nc, cvps_pool, out_pool, w_t, b_t, dst, ch * Lc,
                    batch, final=last,
                )


def _emit_conv_batch(nc, cvps_pool, out_pool, w_t, b_t, dst, l_off, batch,
                     final=False):
    """Conv + bias for a batch of seq positions, 2 l's per PSUM bank.

    All conv matmuls are emitted back-to-back so the W stationary operand
    is swapped in once per batch instead of once per l. ScalarE drains
    each 2-l bank with a contiguous bias-add ACTIVATE into a per-4l
    output tile (its own tile, so stores never WAR-serialize against
    later ACTIVATEs), which is then streamed out full-width to HBM.
    """
    pairs = [batch[i:i + 2] for i in range(0, len(batch), 2)]
    tiles = []
    for pair in pairs:
        cvp = cvps_pool.tile([128, 512], F32, name="cvp")
        for s, (h0s, l0) in enumerate(pair):
            for vh in range(2):
                nc.tensor.matmul(
                    cvp[vh * 64:(vh + 1) * 64, s * 256:(s + 1) * 256],
                    lhsT=w_t[:], rhs=h0s[:, vh * 256:(vh + 1) * 256],
                    start=True, stop=True, tile_position=(0, vh * 64),
                    skip_group_check=True,
                )
        tiles.append((cvp, pair))
    for g in range(0, len(tiles), 2):
        grp = tiles[g:g + 2]
        nl_tot = sum(len(pair) for _, pair in grp)
        o_sb = out_pool.tile([128, nl_tot * 256], F32, name="o_sb")
        off = 0
        for ti, (cvp, pair) in enumerate(grp):
            nl = len(pair)
            # out = in * 1 + bias; ScalarE in steady state, alternating
            # with DVE on the final batch so the serial tail halves.
            if final and ti % 2 == 1:
                nc.vector.tensor_scalar_add(
                    out=o_sb[:, off * 256:(off + nl) * 256],
                    in0=cvp[:, 0:nl * 256],
                    scalar1=b_t[:, 0:1],
                )
            else:
                nc.scalar.add(
                    out=o_sb[:, off * 256:(off + nl) * 256],
                    in_=cvp[:, 0:nl * 256],
                    add=b_t[:, 0:1],
                )
            off += nl
        l_abs = l_off + grp[0][1][0][1]
        nc.sync.dma_start(
            dst[:, l_abs:l_abs + nl_tot, :],
            o_sb[:].rearrange("p (l v) -> p l v", v=256),
        )


def build_nc(cfg: CFG):
    nc = bacc.Bacc("TRN2", target_bir_lowering=False, debug=False)
    ins = {
        "x2": nc.dram_tensor("x2", [N, cfg.L, C_IN], cfg.mm_dt,
                             kind="ExternalInput").ap(),
        "gi": nc.dram_tensor("gi", [NB, 128, GDEP, N], cfg.mm_dt,
                             kind="ExternalInput").ap(),
        "ident": nc.dram_tensor("ident", [128, 128], cfg.mm_dt,
                                kind="ExternalInput").ap(),
        "wt": nc.dram_tensor("wt", [128, C_OUT], cfg.mm_dt,
                             kind="ExternalInput").ap(),
        "bias2": nc.dram_tensor("bias2", [128, 1], F32,
                                kind="ExternalInput").ap(),
    }
    outs = {
        "out": nc.dram_tensor("out", [2, C_OUT, cfg.L, 256], F32,
                              kind="ExternalOutput").ap(),
    }
    with tile.TileContext(nc) as tc:
        body(nc, tc, outs, ins, cfg)
    nc.compile()
    return nc


def make_host_inputs(A, W, b):
    """Precompute the replicated operands: GI (I + G_k^T per node block),
    W^T, bias2."""
    A = np.asarray(A, np.float64)
    n = A.shape[0]
    An = A + np.eye(n)
    An = An / An.sum(axis=1, keepdims=True)
    As = (1.0 - BETA) * An
    eye = np.eye(n)
    G = []
    gk = eye
    for _ in range(GDEP):
        gk = As @ gk + BETA * eye
        G.append(gk)
    import ml_dtypes
    bf16 = ml_dtypes.bfloat16
    # GI[wb, :, k] = G_{k+1}^T block rows
    GI = np.zeros((NB, 128, GDEP, n), dtype=bf16)
    for wb in range(NB):
        rows = slice(wb * 128, (wb + 1) * 128)
        for k in range(GDEP):
            GI[wb, :, k] = G[k].T[rows].astype(bf16)
    GI = np.ascontiguousarray(GI)
    I128 = np.eye(128, dtype=bf16)
    WT = np.ascontiguousarray(np.asarray(W, np.float64).T.astype(bf16))
    b = np.asarray(b, np.float32)
    b2 = np.ascontiguousarray(np.concatenate([b, b]).reshape(128, 1))
    return GI, I128, WT, b2


_NC_CACHE = {}


def run_on_hw(X, A, W, b, cfg=None, trace=False, **spmd_kwargs):
    import ml_dtypes
    bf16 = ml_dtypes.bfloat16
    X = np.asarray(X, np.float32)
    # [B, C, N, L] -> [B, N, L, C] contiguous bf16 for clean device DMA
    X2 = np.ascontiguousarray(X.transpose(0, 2, 3, 1)).astype(bf16)
    GI, I128, WT, b2 = make_host_inputs(A, W, b)
    if cfg is None:
        cfg = CFG()
    key = (cfg.L, cfg.Lc, cfg.mm_dt)
    if key not in _NC_CACHE:
        _NC_CACHE[key] = build_nc(cfg)
    nc = _NC_CACHE[key]
    in_maps = [
        {"x2": X2[i], "gi": GI, "ident": I128, "wt": WT, "bias2": b2}
        for i in range(B)
    ]
    res = bass_utils.run_bass_kernel_spmd(
        nc, in_maps, core_ids=list(range(B)), trace=trace, **spmd_kwargs
    )
    # device out: [2, C_OUT, L, 256] (vh, o, l, v-half) -> [C_OUT, N, L]
    out = np.empty((B, C_OUT, N, L), np.float32)
    for i in range(B):
        o3 = res.results[i]["out"]  # [2, 64, L, 256]
        out[i] = o3.transpose(1, 0, 3, 2).reshape(C_OUT, N, L)
    return out, res


def kernel(X, A, W, b):
    return run_on_hw(X, A, W, b)[0]


if __name__ == "__main__":
    rng = np.random.default_rng(0)
    X = rng.standard_normal((B, C_IN, N, L), dtype=np.float32)
    A = rng.random((N, N), dtype=np.float32)
    W = rng.standard_normal((C_OUT, (GDEP + 1) * C_IN), dtype=np.float32) * 0.1
    b = rng.random(C_OUT, dtype=np.float32)
    out = kernel(X, A, W, b)
    print("out", out.shape, out.dtype, float(np.abs(out).mean()))
